# revision 11
# baseline (speedup 1.0000x reference)
"""Category-specific linear (MoE-routing style) Trainium2 Bass kernel.

Computes out[n] = x[n] @ W[cat_ids[n]] + b[cat_ids[n]] for
x: [N, M, D_IN] f32, cat_ids: [N] int64, W: [C, D_IN, D_H] f32, b: [C, D_H] f32.

Strategy (8-core SPMD, full inputs in / full output out):
  Host: stable-sort samples by category, split into 8 equal shards of
  N/8 samples (perfect load balance).  Within a shard, each category is a
  contiguous run; runs are padded to whole 128-row tiles (8 samples) so the
  device program is fully static.  x rows are pre-transposed on the host
  (fp32 has no DMA-transpose path on TRN2) into [2, 128, NT*128] so the
  contraction dim lands on SBUF partitions.  Each core also gets a small
  deduplicated weight table (its <=KMAX distinct categories) and a per-tile
  weight-slot index.
  Device: the weight table lives in SBUF; for each 128-row tile the weight
  slot index is loaded into a PE register (values_load) and the matmul's
  moving operand is selected with a dynamic slice - zero weight duplication
  in HBM traffic, no dynamic control flow.  Two accumulating matmuls per
  tile (contraction 256 = 2x128), PSUM -> SBUF copy, batched stores.
"""

import os
import sys

import numpy as np

for _p in ("/opt/trn_rl_repo",):
    if os.path.isdir(_p) and _p not in sys.path:
        sys.path.insert(0, _p)

import concourse.bass as bass  # noqa: E402
import concourse.mybir as mybir  # noqa: E402
import concourse.tile as tile  # noqa: E402
from concourse import bacc  # noqa: E402
from concourse.bass import ds  # noqa: E402
from concourse.bass_utils import run_bass_kernel_spmd  # noqa: E402

NCORES = 8
P = 128  # SBUF partitions / rows per tile
D_IN = 256  # contraction dim (2 chunks of 128)
D_H = 256  # output dim
ROWS_PER_SAMPLE = 16
SPT = P // ROWS_PER_SAMPLE  # samples per tile = 8
TB = 8  # tile-count quantum (NT is padded to a multiple of this)
TBI = 16  # tiles per index-register load
OB = 8  # tiles per psum group / out-store DMA

# filled by kernel() for test harness introspection
last_results = None


def _pack(x, cat_ids, W):
    """Host-side routing: sort, shard, pad, transpose, dedup weights.

    Returns (in_maps, scatter_info, NT, KMAX).
    scatter_info[k] = (sample_ids_per_padded_slot [NT*SPT] int64, valid mask)
    """
    N, M, Din = x.shape
    assert M == ROWS_PER_SAMPLE and Din == D_IN
    assert N % NCORES == 0
    S = N // NCORES

    cat = np.asarray(cat_ids).astype(np.int64).ravel()
    order = np.argsort(cat, kind="stable")

    # per-core padded sample lists + per-tile categories
    padded_ids = []  # [NT*SPT] int64, -1 = pad
    tile_cats = []  # list of per-tile category (python list per core)
    for k in range(NCORES):
        idx = order[k * S : (k + 1) * S]
        cats = cat[idx]
        bounds = np.flatnonzero(np.diff(cats)) + 1
        starts = np.concatenate([[0], bounds])
        ends = np.concatenate([bounds, [S]])
        ids_parts = []
        tcats = []
        for s, e in zip(starts, ends):
            c = int(cats[s])
            n = int(e - s)
            npad = (-n) % SPT
            ids_parts.append(idx[s:e])
            if npad:
                ids_parts.append(np.full(npad, -1, np.int64))
            tcats.extend([c] * ((n + npad) // SPT))
        padded_ids.append(np.concatenate(ids_parts))
        tile_cats.append(tcats)

    NT = max(len(t) for t in tile_cats)
    NT = ((NT + TB - 1) // TB) * TB  # whole DMA groups

    # pad every core to NT tiles
    for k in range(NCORES):
        extra = NT - len(tile_cats[k])
        if extra:
            fill_cat = tile_cats[k][0]
            tile_cats[k] = tile_cats[k] + [fill_cat] * extra
            padded_ids[k] = np.concatenate(
                [padded_ids[k], np.full(extra * SPT, -1, np.int64)]
            )

    # per-core weight dedup
    uniq_list = []
    for k in range(NCORES):
        seen = dict()
        for c in tile_cats[k]:
            if c not in seen:
                seen[c] = len(seen)
        uniq_list.append(seen)
    KMAX = max(len(u) for u in uniq_list)

    np_in = _np_in_dtype()
    in_maps = []
    scatter = []
    for k in range(NCORES):
        ids = padded_ids[k]
        valid = ids >= 0
        # gather + zero-pad x rows: [NT*SPT, M, Din]
        Xr = np.zeros((NT * SPT, M, Din), np.float32)
        Xr[valid] = x[ids[valid]]
        # transpose to [Din, NT*P] then chunk the contraction dim
        xT = np.ascontiguousarray(
            Xr.reshape(NT * P, Din).T.astype(np_in)
        ).reshape(2, P, NT * P)

        seen = uniq_list[k]
        w_ids = list(seen.keys())
        w_ids += [w_ids[0]] * (KMAX - len(w_ids))
        Wp = W[np.asarray(w_ids, np.int64)]  # [KMAX, Din, D_H]
        Wl = np.ascontiguousarray(
            Wp.reshape(KMAX, 2, P, D_H).transpose(2, 1, 0, 3).astype(np_in)
        )  # [P, 2, KMAX, D_H]

        widx = np.asarray([seen[c] for c in tile_cats[k]], np.int32)[None, :]

        in_maps.append({"xT": xT, "Wl": Wl, "widx": widx})
        scatter.append((ids, valid))

    return in_maps, scatter, NT, KMAX


def _dt_mode():
    return os.environ.get("CSL_DT_MODE", "f16")


def _out_mode():
    return os.environ.get("CSL_OUT_DT", "f32")


def _np_in_dtype():
    import ml_dtypes

    return {
        "f16": np.float16,
        "bf16": ml_dtypes.bfloat16,
        "f32r": np.float32,
        "f32": np.float32,
    }[_dt_mode()]


def _mm_dt():
    return {
        "f16": mybir.dt.float16,
        "bf16": mybir.dt.bfloat16,
        "f32r": mybir.dt.float32r,
        "f32": mybir.dt.float32,
    }[_dt_mode()]


def _build(NT, KMAX):
    """Build the SPMD device program for NT tiles and KMAX weight slots."""
    mm_dt = _mm_dt()
    out_dt = mybir.dt.float32 if _out_mode() == "f32" else mybir.dt.float16
    f32 = mybir.dt.float32
    i32 = mybir.dt.int32
    static_idx = os.environ.get("CSL_STATIC", "0") == "1"

    nc = bacc.Bacc(
        "TRN2",
        target_bir_lowering=False,
        debug=False,
        enable_asserts=False,
        num_devices=NCORES,
    )
    NTR = NT * P
    GX = 24  # tiles per x-load DMA group
    xT_d = nc.dram_tensor("xT", [2, P, NTR], mm_dt, kind="ExternalInput").ap()
    W_d = nc.dram_tensor("Wl", [P, 2, KMAX, D_H], mm_dt, kind="ExternalInput").ap()
    wi_d = nc.dram_tensor("widx", [1, NT], i32, kind="ExternalInput").ap()
    out_d = nc.dram_tensor("out", [NTR, D_H], out_dt, kind="ExternalOutput").ap()
    out_v = out_d.rearrange("(n p) h -> p n h", p=P)

    with tile.TileContext(nc) as tc:
        with (
            tc.tile_pool(name="wpool", bufs=1) as wpool,
            tc.tile_pool(name="xpool", bufs=3) as xpool,
            tc.tile_pool(name="opool", bufs=3) as opool,
            tc.tile_pool(name="psum", bufs=2, space="PSUM") as psum_pool,
        ):
            # widx first (tiny, unblocks index loads); W on the Scalar ring
            # so it issues in parallel with the Sync-ring x loads
            wi_sb = wpool.tile([1, NT], i32)
            nc.sync.dma_start(wi_sb[:], wi_d)
            W_sb = wpool.tile([P, 2, KMAX, D_H], mm_dt)
            nc.scalar.dma_start(W_sb[:], W_d)

            for g0 in range(0, NT, GX):
                gx = min(GX, NT - g0)
                # loads on the Sync HWDGE ring; stores go on the Scalar ring
                # so a store waiting on DVE never blocks a prefetch load
                xt = xpool.tile([P, 2, GX * P], mm_dt)
                nc.sync.dma_start(
                    xt[:, 0, : gx * P], xT_d[0, :, g0 * P : (g0 + gx) * P]
                )
                nc.sync.dma_start(
                    xt[:, 1, : gx * P], xT_d[1, :, g0 * P : (g0 + gx) * P]
                )
                for i0 in range(0, gx, TBI):
                    ti = min(TBI, gx - i0)
                    if static_idx:
                        vals = (0,) * ti  # debug: no dynamic indexing
                    else:
                        # one TENSOR_LOAD for ti per-tile weight slots
                        _, vals = nc.values_load_multi_w_load_instructions(
                            wi_sb[0:1, g0 + i0 : g0 + i0 + ti],
                            engines=(mybir.EngineType.PE,),
                            min_val=0,
                            max_val=KMAX - 1,
                            skip_runtime_bounds_check=True,
                        )
                    for o0 in range(0, ti, OB):
                        ps = psum_pool.tile([P, OB, D_H], f32)
                        ot = opool.tile([P, OB, D_H], out_dt)
                        for j in range(OB):
                            tt = i0 + o0 + j  # tile within group
                            widx = vals[o0 + j]
                            nc.tensor.matmul(
                                ps[:, j, :],
                                xt[:, 0, tt * P : (tt + 1) * P],
                                W_sb[:, 0, ds(widx, 1), :],
                                start=True,
                                stop=False,
                            )
                            nc.tensor.matmul(
                                ps[:, j, :],
                                xt[:, 1, tt * P : (tt + 1) * P],
                                W_sb[:, 1, ds(widx, 1), :],
                                start=False,
                                stop=True,
                            )
                        nc.vector.tensor_copy(ot[:], ps[:])
                        t_abs = g0 + i0 + o0
                        nc.scalar.dma_start(
                            out_v[:, t_abs : t_abs + OB, :], ot[:]
                        )

    nc.compile()
    return nc


def kernel(x=None, cat_ids=None, W=None, b=None, **_unused):
    global last_results
    x = np.asarray(x, np.float32)
    W = np.asarray(W, np.float32)
    N, M, _ = x.shape

    in_maps, scatter, NT, KMAX = _pack(x, cat_ids, W)

    nc = _build(NT, KMAX)

    trace = os.environ.get("CSL_TRACE", "0") == "1"
    kwargs = {}
    if trace:
        kwargs["trace"] = True
        tc_env = os.environ.get("CSL_TRACE_CORES", "")
        if tc_env:
            kwargs["trace_cores"] = [int(c) for c in tc_env.split(",")]
        else:
            kwargs["trace_cores"] = list(range(NCORES))
    res = run_bass_kernel_spmd(
        nc, in_maps, core_ids=list(range(NCORES)), **kwargs
    )
    last_results = res

    out = np.empty((N, M, D_H), np.float32)
    for k in range(NCORES):
        ids, valid = scatter[k]
        ok = res.results[k]["out"].astype(np.float32, copy=False)
        ok = ok.reshape(NT * SPT, ROWS_PER_SAMPLE, D_H)
        out[ids[valid]] = ok[valid]

    if b is not None:
        b = np.asarray(b, np.float32)
        if np.any(b):
            cat = np.asarray(cat_ids).astype(np.int64).ravel()
            out += b[cat][:, None, :]

    return out


# revision 15
# speedup vs baseline: 1.2845x; 1.2845x over previous
"""Category-specific linear (MoE-routing style) Trainium2 Bass kernel.

Computes out[n] = x[n] @ W[cat_ids[n]] + b[cat_ids[n]] for
x: [N, M, D_IN] f32, cat_ids: [N] int64, W: [C, D_IN, D_H] f32, b: [C, D_H] f32.

Strategy (8-core SPMD, full inputs in / full output out):
  Host: stable-sort samples by category, split into 8 equal shards of
  N/8 samples (perfect load balance).  Within a shard, each category is a
  contiguous run; runs are padded to whole 128-row tiles (8 samples) so the
  device program is fully static.  x rows are pre-transposed on the host
  (fp32 has no DMA-transpose path on TRN2) into [2, 128, NT*128] so the
  contraction dim lands on SBUF partitions.  Each core also gets a small
  deduplicated weight table (its <=KMAX distinct categories) and a per-tile
  weight-slot index.
  Device: the weight table lives in SBUF; for each 128-row tile the weight
  slot index is loaded into a PE register (values_load) and the matmul's
  moving operand is selected with a dynamic slice - zero weight duplication
  in HBM traffic, no dynamic control flow.  Two accumulating matmuls per
  tile (contraction 256 = 2x128), PSUM -> SBUF copy, batched stores.
"""

import os
import sys

import numpy as np

for _p in ("/opt/trn_rl_repo",):
    if os.path.isdir(_p) and _p not in sys.path:
        sys.path.insert(0, _p)

import concourse.bass as bass  # noqa: E402
import concourse.mybir as mybir  # noqa: E402
import concourse.tile as tile  # noqa: E402
from concourse import bacc  # noqa: E402
from concourse.bass import ds  # noqa: E402
from concourse.bass_utils import run_bass_kernel_spmd  # noqa: E402

NCORES = 8
P = 128  # SBUF partitions / rows per tile
D_IN = 256  # contraction dim (2 chunks of 128)
D_H = 256  # output dim
ROWS_PER_SAMPLE = 16
SPT = P // ROWS_PER_SAMPLE  # samples per tile = 8
TB = 8  # tile-count quantum (NT is padded to a multiple of this)
TBI = 16  # tiles per index-register load
OB = 4  # tiles per psum group / DVE copy
OS = 8  # tiles per out-store DMA

# filled by kernel() for test harness introspection
last_results = None


def _pack(x, cat_ids, W):
    """Host-side routing: sort, shard, pad, transpose, dedup weights.

    Returns (in_maps, scatter_info, NT, KMAX).
    scatter_info[k] = (sample_ids_per_padded_slot [NT*SPT] int64, valid mask)
    """
    N, M, Din = x.shape
    assert M == ROWS_PER_SAMPLE and Din == D_IN
    assert N % NCORES == 0
    S = N // NCORES

    cat = np.asarray(cat_ids).astype(np.int64).ravel()
    order = np.argsort(cat, kind="stable")

    # per-core padded sample lists + per-tile categories
    padded_ids = []  # [NT*SPT] int64, -1 = pad
    tile_cats = []  # list of per-tile category (python list per core)
    for k in range(NCORES):
        idx = order[k * S : (k + 1) * S]
        cats = cat[idx]
        bounds = np.flatnonzero(np.diff(cats)) + 1
        starts = np.concatenate([[0], bounds])
        ends = np.concatenate([bounds, [S]])
        ids_parts = []
        tcats = []
        for s, e in zip(starts, ends):
            c = int(cats[s])
            n = int(e - s)
            npad = (-n) % SPT
            ids_parts.append(idx[s:e])
            if npad:
                ids_parts.append(np.full(npad, -1, np.int64))
            tcats.extend([c] * ((n + npad) // SPT))
        padded_ids.append(np.concatenate(ids_parts))
        tile_cats.append(tcats)

    NT = max(len(t) for t in tile_cats)
    NT = ((NT + TB - 1) // TB) * TB  # whole DMA groups

    # pad every core to NT tiles
    for k in range(NCORES):
        extra = NT - len(tile_cats[k])
        if extra:
            fill_cat = tile_cats[k][0]
            tile_cats[k] = tile_cats[k] + [fill_cat] * extra
            padded_ids[k] = np.concatenate(
                [padded_ids[k], np.full(extra * SPT, -1, np.int64)]
            )

    # per-core weight dedup
    uniq_list = []
    for k in range(NCORES):
        seen = dict()
        for c in tile_cats[k]:
            if c not in seen:
                seen[c] = len(seen)
        uniq_list.append(seen)
    KMAX = max(len(u) for u in uniq_list)

    np_in = _np_in_dtype()
    in_maps = []
    scatter = []
    for k in range(NCORES):
        ids = padded_ids[k]
        valid = ids >= 0
        # gather + zero-pad x rows: [NT*SPT, M, Din]
        Xr = np.zeros((NT * SPT, M, Din), np.float32)
        Xr[valid] = x[ids[valid]]
        # transpose to [Din, NT*P] then chunk the contraction dim
        xT = np.ascontiguousarray(
            Xr.reshape(NT * P, Din).T.astype(np_in)
        ).reshape(2, P, NT * P)

        seen = uniq_list[k]
        w_ids = list(seen.keys())
        w_ids += [w_ids[0]] * (KMAX - len(w_ids))
        Wp = W[np.asarray(w_ids, np.int64)]  # [KMAX, Din, D_H]
        Wl = np.ascontiguousarray(
            Wp.reshape(KMAX, 2, P, D_H).transpose(2, 1, 0, 3).astype(np_in)
        )  # [P, 2, KMAX, D_H]

        widx = np.asarray([seen[c] for c in tile_cats[k]], np.int32)[None, :]

        in_maps.append({"xT": xT, "Wl": Wl, "widx": widx})
        scatter.append((ids, valid))

    return in_maps, scatter, NT, KMAX


def _dt_mode():
    return os.environ.get("CSL_DT_MODE", "f16")


def _out_mode():
    return os.environ.get("CSL_OUT_DT", "f16")


def _np_in_dtype():
    import ml_dtypes

    return {
        "f16": np.float16,
        "bf16": ml_dtypes.bfloat16,
        "f32r": np.float32,
        "f32": np.float32,
    }[_dt_mode()]


def _mm_dt():
    return {
        "f16": mybir.dt.float16,
        "bf16": mybir.dt.bfloat16,
        "f32r": mybir.dt.float32r,
        "f32": mybir.dt.float32,
    }[_dt_mode()]


def _build(NT, KMAX):
    """Build the SPMD device program for NT tiles and KMAX weight slots."""
    mm_dt = _mm_dt()
    out_dt = mybir.dt.float32 if _out_mode() == "f32" else mybir.dt.float16
    f32 = mybir.dt.float32
    i32 = mybir.dt.int32
    static_idx = os.environ.get("CSL_STATIC", "0") == "1"

    nc = bacc.Bacc(
        "TRN2",
        target_bir_lowering=False,
        debug=False,
        enable_asserts=False,
        num_devices=NCORES,
    )
    NTR = NT * P
    GX = 16  # tiles per x-load DMA group
    xT_d = nc.dram_tensor("xT", [2, P, NTR], mm_dt, kind="ExternalInput").ap()
    W_d = nc.dram_tensor("Wl", [P, 2, KMAX, D_H], mm_dt, kind="ExternalInput").ap()
    wi_d = nc.dram_tensor("widx", [1, NT], i32, kind="ExternalInput").ap()
    # partition-major output layout: fully contiguous per-partition stores;
    # the host untransposes when scattering back
    out_d = nc.dram_tensor("out", [P, NT, D_H], out_dt, kind="ExternalOutput").ap()

    with tile.TileContext(nc) as tc:
        with (
            tc.tile_pool(name="wpool", bufs=1) as wpool,
            tc.tile_pool(name="xpool", bufs=3) as xpool,
            tc.tile_pool(name="opool", bufs=3) as opool,
            tc.tile_pool(name="psum", bufs=4, space="PSUM") as psum_pool,
        ):
            # widx first (tiny, unblocks index loads); W on the Scalar ring
            # so it issues in parallel with the Sync-ring x loads
            wi_sb = wpool.tile([1, NT], i32)
            nc.sync.dma_start(wi_sb[:], wi_d)
            W_sb = wpool.tile([P, 2, KMAX, D_H], mm_dt)
            nc.scalar.dma_start(W_sb[:], W_d)

            for g0 in range(0, NT, GX):
                gx = min(GX, NT - g0)
                # loads on the Sync HWDGE ring; stores go on the Scalar ring
                # so a store waiting on DVE never blocks a prefetch load
                xt = xpool.tile([P, 2, GX * P], mm_dt)
                nc.sync.dma_start(
                    xt[:, 0, : gx * P], xT_d[0, :, g0 * P : (g0 + gx) * P]
                )
                nc.sync.dma_start(
                    xt[:, 1, : gx * P], xT_d[1, :, g0 * P : (g0 + gx) * P]
                )
                for i0 in range(0, gx, TBI):
                    ti = min(TBI, gx - i0)
                    if static_idx:
                        vals = (0,) * ti  # debug: no dynamic indexing
                    else:
                        # one TENSOR_LOAD for ti per-tile weight slots
                        _, vals = nc.values_load_multi_w_load_instructions(
                            wi_sb[0:1, g0 + i0 : g0 + i0 + ti],
                            engines=(mybir.EngineType.PE,),
                            min_val=0,
                            max_val=KMAX - 1,
                            skip_runtime_bounds_check=True,
                        )
                    for s0 in range(0, ti, OS):
                        ot = opool.tile([P, OS, D_H], out_dt)
                        for o0 in range(s0, s0 + OS, OB):
                            ps = psum_pool.tile([P, OB, D_H], f32)
                            for j in range(OB):
                                tt = i0 + o0 + j  # tile within group
                                widx = vals[o0 + j]
                                nc.tensor.matmul(
                                    ps[:, j, :],
                                    xt[:, 0, tt * P : (tt + 1) * P],
                                    W_sb[:, 0, ds(widx, 1), :],
                                    start=True,
                                    stop=False,
                                )
                                nc.tensor.matmul(
                                    ps[:, j, :],
                                    xt[:, 1, tt * P : (tt + 1) * P],
                                    W_sb[:, 1, ds(widx, 1), :],
                                    start=False,
                                    stop=True,
                                )
                            nc.vector.tensor_copy(
                                ot[:, o0 - s0 : o0 - s0 + OB], ps[:]
                            )
                        t_abs = g0 + i0 + s0
                        nc.scalar.dma_start(
                            out_d[:, t_abs : t_abs + OS, :], ot[:]
                        )

    nc.compile()
    return nc


def kernel(x=None, cat_ids=None, W=None, b=None, **_unused):
    global last_results
    x = np.asarray(x, np.float32)
    W = np.asarray(W, np.float32)
    N, M, _ = x.shape

    in_maps, scatter, NT, KMAX = _pack(x, cat_ids, W)

    nc = _build(NT, KMAX)

    trace = os.environ.get("CSL_TRACE", "0") == "1"
    kwargs = {}
    if trace:
        kwargs["trace"] = True
        tc_env = os.environ.get("CSL_TRACE_CORES", "")
        if tc_env:
            kwargs["trace_cores"] = [int(c) for c in tc_env.split(",")]
        else:
            kwargs["trace_cores"] = list(range(NCORES))
    res = run_bass_kernel_spmd(
        nc, in_maps, core_ids=list(range(NCORES)), **kwargs
    )
    last_results = res

    out = np.empty((N, M, D_H), np.float32)
    for k in range(NCORES):
        ids, valid = scatter[k]
        # device layout [P, NT, D_H] -> row-major [NT*P, D_H]
        ok = res.results[k]["out"].astype(np.float32, copy=False)
        ok = ok.transpose(1, 0, 2).reshape(NT * SPT, ROWS_PER_SAMPLE, D_H)
        out[ids[valid]] = ok[valid]

    if b is not None:
        b = np.asarray(b, np.float32)
        if np.any(b):
            cat = np.asarray(cat_ids).astype(np.int64).ravel()
            out += b[cat][:, None, :]

    return out
